# revision 25
# baseline (speedup 1.0000x reference)
"""DCNv4 Trainium2 kernel — data-parallel over batch (1 sample per NeuronCore).

Math reformulation (validated vs reference in numpy, absmax ~1.3e-3 fp16):
  The bilinear grid_sample at ref+grid+offset positions with |offset| < 1 is an
  exact 3-tap "hat" stencil per point: weight on pixel (base+d) = max(0, 1-|t-d|),
  d in {-1,0,1}, t = learned offset.  Folding the 9 points x 3x3 taps by total
  shift gives 25 weight planes WW[g, dy, dx, l]; the output is
     out[c, l] = sum_{dy,dx} WW[g(c), dy, dx, l] * V2[c, l + (dy,dx)]
  over a zero-halo'd value image V2 (68x68).  Everything else is plain matmuls.

Layout: (channel partitions, location free).  Per-core phases:
  A  DMA x (c,l) + x row-view (f16 cast)
  B  PE transposes -> x2T padded table (om-branch input)
  C  PE v = vp_w@x+b -> V2 halo table (f16)
  D  PE depthwise 3x3 via diagonal-stationary matmuls
  E  PE om = om_w_perm @ dw + b  (rows pre-permuted into tx|ty|m fields)
  F  DVE hats h = -(min(|t-d|,1)-1), products T_ab = (hy_a*m)*hx_b (signs cancel)
  G  PE 0/1-matrix reduction T_ab -> 25x8 WW planes; stage to DRAM per L-half
  H  DMA broadcast WW rows across group channels (sync+act HWDGE rings);
     DVE multiplies U_d = we_d * V2shift; PE accumulates the 25 planes into
     PSUM via identity-stationary matmuls (f32 accumulate); Act drains to SBUF.
     Split into two L-halves so half-0 broadcast/compute overlaps G half-1.
  I  PE final projection op_w @ acc + b -> out
"""

import sys

sys.path.insert(0, "/opt/trn_rl_repo")

from contextlib import ExitStack

import numpy as np

import concourse.bass as bass
import concourse.mybir as mybir
import concourse.tile as tile
from concourse.bass_utils import run_bass_kernel_spmd
from concourse.masks import make_identity

F32 = mybir.dt.float32
F16 = mybir.dt.float16
ALU = mybir.AluOpType
ACT = mybir.ActivationFunctionType

N, C, H, W = 8, 256, 64, 64
G, GC, L = 8, 32, 4096
HALO = 68  # 64 + 2 pad + 2 hat-halo
NCORES = 8
LH = L // 2  # per-half columns (rows 0-31 / 32-63 of the 64x64 image)


def _om_perm():
    perm = np.zeros(216, np.int64)
    for p in range(9):
        for g in range(8):
            j = 8 * p + g
            perm[j] = 27 * g + 2 * p
            perm[72 + j] = 27 * g + 2 * p + 1
            perm[144 + j] = 27 * g + 18 + p
    return perm


def _s_matrices():
    # S[a,b] maps T-field partitions (8p+g) -> plane partitions q=8*(5*(ky+a)+(kx+b))+g
    S = np.zeros((3, 3, 72, 200), np.float32)
    for a in range(3):
        for b in range(3):
            for p in range(9):
                kx, ky = p // 3, p % 3
                didx = 5 * (ky + a) + (kx + b)
                for g in range(8):
                    S[a, b, 8 * p + g, 8 * didx + g] = 1.0
    return S


def _ap(base, offset_elems, pattern):
    return bass.AP(tensor=base.tensor, offset=base.offset + offset_elems, ap=pattern)


def _split_sync_waits(nc, maxw=1):
    """This container's walrus rejects instructions with more than ~1 sync
    wait (e.g. any 3D-AP Activation with 2 waits, or Tile's tail drain with
    one wait per live semaphore).  Hoist excess waits onto same-engine NoOps
    placed immediately before the instruction — identical blocking semantics,
    since waits are monotone pre-conditions executed in queue order."""
    nsplit = 0
    for f in nc.m.functions:
        for blk in f.blocks:
            il = blk.instructions
            new, changed = [], False
            for ins in il:
                si = ins.sync_info
                if si is not None and si.on_wait and len(si.on_wait) > maxw:
                    waits = list(si.on_wait)
                    for j, w in enumerate(waits[:-maxw]):
                        nop = mybir.InstNoOp(name=f"{ins.name}-ws{j}", ins=[], outs=[])
                        nop.engine = ins.engine
                        nop.sync_info = mybir.SyncInfo(on_wait=[w], on_update=[])
                        new.append(nop)
                        nsplit += 1
                    ins.sync_info = mybir.SyncInfo(
                        on_wait=waits[-maxw:], on_update=list(si.on_update)
                    )
                    changed = True
                new.append(ins)
            if changed:
                blk.instructions = new
    return nsplit


def build_module(split_waits=True):
    nc = bass.Bass()

    x_e = nc.declare_dram_parameter("x", [C, L], F32, isOutput=False)
    out_e = nc.declare_dram_parameter("out", [C, L], F32, isOutput=True)
    vpw_e = nc.declare_dram_parameter("vp_wt", [C, C], F16, isOutput=False)
    vpb_e = nc.declare_dram_parameter("vp_b2", [128, 2], F32, isOutput=False)
    ddg_e = nc.declare_dram_parameter("dw_diag", [128, 18, 128], F16, isOutput=False)
    dwb_e = nc.declare_dram_parameter("dw_b2", [128, 2], F32, isOutput=False)
    omw_e = nc.declare_dram_parameter("om_wt", [C, 216], F16, isOutput=False)
    omb_e = nc.declare_dram_parameter("om_b3", [72, 3], F32, isOutput=False)
    sm_e = nc.declare_dram_parameter("smat", [72, 18, 128], F16, isOutput=False)
    opw_e = nc.declare_dram_parameter("op_wt", [C, C], F16, isOutput=False)
    opb_e = nc.declare_dram_parameter("op_b2", [128, 2], F32, isOutput=False)
    exs_e = nc.declare_dram_parameter("exsel", [128, 8, 128], F16, isOutput=False)

    with tile.TileContext(nc) as tc, ExitStack() as es:
        cpool = es.enter_context(tc.tile_pool(name="consts", bufs=1))
        big = es.enter_context(tc.tile_pool(name="big", bufs=1))
        pp = es.enter_context(tc.tile_pool(name="pp", bufs=1, space="PSUM"))
        wk = es.enter_context(tc.tile_pool(name="work", bufs=1))
        wxp = es.enter_context(tc.tile_pool(name="wexp", bufs=8))
        upool = es.enter_context(tc.tile_pool(name="upool", bufs=3))
        dpool = es.enter_context(tc.tile_pool(name="dram", bufs=1, space="DRAM"))

        # ---- constants -------------------------------------------------
        vpw = [cpool.tile([128, C], F16, name=f"vpw{k}", tag=f"vpw{k}") for k in range(2)]
        for k in range(2):
            nc.sync.dma_start(out=vpw[k], in_=vpw_e[128 * k : 128 * (k + 1), :])
        vpb = cpool.tile([128, 2], F32)
        nc.sync.dma_start(out=vpb, in_=vpb_e[:, :])
        ddg = cpool.tile([128, 18, 128], F16)
        nc.sync.dma_start(out=ddg, in_=ddg_e[:, :, :])
        dwb = cpool.tile([128, 2], F32)
        nc.sync.dma_start(out=dwb, in_=dwb_e[:, :])
        omw = [cpool.tile([128, 216], F16, name=f"omw{k}", tag=f"omw{k}") for k in range(2)]
        for k in range(2):
            nc.scalar.dma_start(out=omw[k], in_=omw_e[128 * k : 128 * (k + 1), :])
        omb = cpool.tile([72, 3], F32)
        nc.scalar.dma_start(out=omb, in_=omb_e[:, :])
        sm = cpool.tile([72, 18, 128], F16)
        nc.scalar.dma_start(out=sm, in_=sm_e[:, :, :])
        opw = [cpool.tile([128, C], F16, name=f"opw{k}", tag=f"opw{k}") for k in range(2)]
        for k in range(2):
            nc.scalar.dma_start(out=opw[k], in_=opw_e[128 * k : 128 * (k + 1), :])
        opb = cpool.tile([128, 2], F32)
        nc.scalar.dma_start(out=opb, in_=opb_e[:, :])
        exs = cpool.tile([128, 8, 128], F16)
        nc.sync.dma_start(out=exs, in_=exs_e[:, :, :])

        # ---- A: input loads (cast f32 -> f16 via SWDGE) ----------------
        # xrow first: it gates the om branch (B->D->E->F->G->broadcast),
        # which is the critical path.  xsb only feeds C (consumed in the
        # H window).
        xrow = big.tile([128, 32, 256], F16, tag="xrow")
        for q4 in range(4):
            nc.gpsimd.dma_start(
                out=xrow[:, 8 * q4 : 8 * (q4 + 1), :],
                in_=_ap(x_e[:, :], 32768 * 8 * q4, [[256, 128], [32768, 8], [1, 256]]),
            )
        xsb = [big.tile([128, L], F16, name=f"xsb{k}", tag=f"xsb{k}") for k in range(2)]
        for k in range(2):
            nc.gpsimd.dma_start(out=xsb[k], in_=x_e[128 * k : 128 * (k + 1), :])

        # ---- B: PE transposes -> x2d (256, 66, 66) padded table --------
        x2tp = [
            big.tile([128, 66, 66], F16, name=f"x2tp{k}", tag=f"x2d{k}")
            for k in range(2)
        ]
        for k in range(2):
            nc.scalar.memzero(x2tp[k])
        ident = cpool.tile([128, 128], F16)
        make_identity(nc, ident)
        # batch 4 transposes into one [128,512] psum bank -> 1 copy (8 rows)
        for ct in range(2):
            for i4 in range(8):
                ps = pp.tile([128, 512], F16, name="tps", tag=f"p{i4 % 4}")
                for s in range(4):
                    i = 4 * i4 + s
                    nc.tensor.transpose(
                        ps[:, 128 * s : 128 * (s + 1)],
                        xrow[:, i, 128 * ct : 128 * (ct + 1)],
                        ident,
                    )
                nc.scalar.copy(
                    out=x2tp[ct][:, 8 * i4 + 1 : 8 * i4 + 9, 1:65],
                    in_=ps[:].rearrange("p (a b) -> p a b", a=8),
                )

        # ---- D: depthwise conv (diagonal stationary, padded input) ----
        dwsb = []
        dwsb.append(big.tile([128, L], F16, name="dwsb0", tag="dwsb0"))
        dwsb.append(big.tile([128, L], F16, name="dwsb1", tag="dwsb1"))
        for ct in range(2):
            banks = [
                pp.tile([128, 512], F32, name=f"dwp{b}", tag=(f"dw{b}" if b < 4 else f"p{b - 4}"))
                for b in range(8)
            ]
            for tap in range(9):
                kh, kw = tap // 3, tap % 3
                lhsT = ddg[:, 2 * tap + ct, :]
                for ch in range(8):
                    rhs = x2tp[ct][:, 8 * ch + kh : 8 * ch + kh + 8, kw : kw + 64]
                    nc.tensor.matmul(
                        banks[ch], lhsT, rhs, start=(tap == 0), stop=(tap == 8)
                    )
            for ch in range(8):
                nc.scalar.activation(
                    out=dwsb[ct][:, 512 * ch : 512 * (ch + 1)],
                    in_=banks[ch],
                    func=ACT.Identity,
                    bias=dwb[:, ct : ct + 1],
                )

        # ---- C: v matmul -> V2 halo table (f16); runs in the H window --
        v2 = [
            big.tile([128, HALO, HALO], F16, name=f"v2_{k}", tag=f"v2_{k}")
            for k in range(2)
        ]
        # zero only the halo border strips (rows 0:2, 66:68; cols 0:2, 66:68)
        for k in range(2):
            nc.vector.memset(v2[k][:, 0:2, :], 0.0)
            nc.vector.memset(v2[k][:, 66:68, :], 0.0)
            nc.vector.memset(v2[k][:, 2:66, 0:2], 0.0)
            nc.vector.memset(v2[k][:, 2:66, 66:68], 0.0)
        for ct in range(2):
            for ch in range(8):
                ps = pp.tile([128, 512], F32, name="vps", tag=f"p{ch % 4}")
                for kt in range(2):
                    nc.tensor.matmul(
                        ps,
                        vpw[kt][:, 128 * ct : 128 * (ct + 1)],
                        xsb[kt][:, 512 * ch : 512 * (ch + 1)],
                        start=(kt == 0),
                        stop=(kt == 1),
                    )
                src = ps[:].rearrange("p (a b) -> p a b", a=8)
                nc.scalar.activation(
                    out=v2[ct][:, 2 + 8 * ch : 10 + 8 * ch, 2:66],
                    in_=src,
                    func=ACT.Identity,
                    bias=vpb[:, ct : ct + 1],
                )

        # ---- E: om matmul -> tx, ty, m fields (f16) -------------------
        tfld = []
        tfld.append(big.tile([72, L], F16, name="tf0", tag="xsb0"))
        tfld.append(big.tile([72, L], F16, name="tf1", tag="xsb1"))
        tfld.append(big.tile([72, L], F16, name="tf2", tag="tf2"))
        for f in range(3):
            for ch in range(8):
                ps = pp.tile([72, 512], F32, name="omps", tag=f"p{ch % 4}")
                for kt in range(2):
                    nc.tensor.matmul(
                        ps,
                        omw[kt][:, 72 * f : 72 * (f + 1)],
                        dwsb[kt][:, 512 * ch : 512 * (ch + 1)],
                        start=(kt == 0),
                        stop=(kt == 1),
                    )
                nc.scalar.activation(
                    out=tfld[f][:, 512 * ch : 512 * (ch + 1)],
                    in_=ps,
                    func=ACT.Identity,
                    bias=omb[:, f : f + 1],
                )

        # ---- F: hats, products T_ab (DVE) -----------------------------
        negd = []
        for di, dv in enumerate((-1.0, 0.0, 1.0)):
            cb = cpool.tile([72, 1], F32, name=f"negd{di}", tag=f"negd{di}")
            nc.vector.memset(cb, -dv)
            negd.append(cb)
        CH = 512
        # ---- F+G interleaved per chunk: products then S-reduction ------
        ww0 = big.tile([128, L], F16, tag="ww0")  # planes q = 0..127
        ww1 = big.tile([72, L], F16, name="ww1", tag="dwsb1")  # planes q = 128..199
        wwd = dpool.tile([200, L], F16, name="wwd")
        for c4 in range(8):
            sl = slice(CH * c4, CH * (c4 + 1))
            hx, hy = [], []
            for di, dv in enumerate((-1.0, 0.0, 1.0)):
                for fld, lst in ((0, hx), (1, hy)):
                    u = wk.tile([72, CH], F16, name=f"u{fld}{di}", tag="u")
                    nc.scalar.activation(
                        out=u, in_=tfld[fld][:, sl], func=ACT.Abs,
                        bias=negd[di][:, 0:1], scale=1.0,
                    )
                    h = wk.tile([72, CH], F16, name=f"h{fld}{di}", tag=f"h{fld}{di}")
                    nc.vector.tensor_scalar(h, u, 1.0, 1.0, ALU.min, ALU.subtract)
                    lst.append(h)
            hm = []
            for a in range(3):
                t = wk.tile([72, CH], F16, name=f"hm{a}", tag=f"hm{a}")
                nc.vector.tensor_tensor(t, hy[a], tfld[2][:, sl], ALU.mult)
                hm.append(t)
            tabs = []
            for a in range(3):
                for b in range(3):
                    t = wk.tile([72, CH], F16, name=f"tab{a}{b}", tag=f"tab{a}{b}")
                    # split products DVE/GpSimd to free DVE for H mults sooner
                    eng = nc.gpsimd if (3 * a + b) % 2 == 1 else nc.vector
                    eng.tensor_tensor(t, hm[a], hx[b], ALU.mult)
                    tabs.append(t)
            for mt, (rows, wwt) in enumerate(((128, ww0), (72, ww1))):
                ps = pp.tile([rows, 512], F32, name="swp", tag=f"p{(2 * c4 + mt) % 4}")
                for ab in range(9):
                    nc.tensor.matmul(
                        ps,
                        sm[:, 2 * ab + mt, :rows],
                        tabs[ab][:],
                        start=(ab == 0),
                        stop=(ab == 8),
                    )
                nc.scalar.copy(out=wwt[:, sl], in_=ps)
                off = 128 * mt
                nc.gpsimd.dma_start(
                    out=wwd[off : off + rows, sl], in_=wwt[:rows, sl]
                )

        # ---- H: aggregation, two L-halves ------------------------------
        # per (half, ct): 25 didx planes.  Broadcast we_d on 3 DMA rings.
        # DVE multiplies U_d = we_d * V2shift.  Adds split: didx < ADD_DVE
        # chain on DVE into the acc region; didx >= ADD_DVE accumulate on PE
        # in PSUM; the DVE chain is folded in by one final identity matmul.
        ADD_DVE = 9  # didx 1..ADD_DVE-1 accumulate on DVE; the rest on PE
        POOL_MULT = set()  # mults on gpsimd (PE-add stream)
        PB_SET = {5, 11, 17, 23}  # we via PE-select, not DMA (interleaved)
        acc = []
        acc.append(big.tile([128, 64, 64], F16, name="acc0", tag="x2d0"))
        acc.append(big.tile([128, 64, 64], F16, name="acc1", tag="x2d1"))
        bceng = [nc.sync, nc.scalar, nc.sync, nc.scalar]

        def emit_I(ch_range):
            for ct in range(2):
                for ch in ch_range:
                    ps = pp.tile([128, 512], F32, name="ops", tag=f"p{2 + ch % 2}")
                    for kt in range(2):
                        rhs = acc[kt][:, 8 * ch : 8 * (ch + 1), :]
                        nc.tensor.matmul(
                            ps,
                            opw[kt][:, 128 * ct : 128 * (ct + 1)],
                            rhs,
                            start=(kt == 0),
                            stop=(kt == 1),
                        )
                    ob = wk.tile([128, 512], F32, name="osb", tag="osb")
                    nc.scalar.activation(
                        out=ob, in_=ps, func=ACT.Identity, bias=opb[:, ct : ct + 1]
                    )
                    nc.sync.dma_start(
                        out=out_e[
                            128 * ct : 128 * (ct + 1), 512 * ch : 512 * (ch + 1)
                        ],
                        in_=ob,
                    )

        for half in range(2):
            for ct in range(2):
                areg = acc[ct][:, 32 * half : 32 * half + 32, :]
                hps = [
                    pp.tile([128, 512], F32, name=f"hps{b}", tag=f"dw{b}")
                    for b in range(4)
                ]
                for d in range(25):
                    dy, dx = d // 5, d % 5
                    we = wxp.tile([128, 32, 64], F16, name="we", tag="we")
                    if d in PB_SET:
                        # PE-select broadcast: we[c,:] = wwt[qb + c//32, :]
                        qb = 8 * d + 4 * ct
                        wwt, rows = (ww0, 128) if d < 16 else (ww1, 72)
                        if d >= 16:
                            qb -= 128
                        a, s = qb // 32, qb % 32
                        kk = min(32, rows - 32 * a)
                        lhsT = exs[32 * a : 32 * a + kk, s // 4, :]
                        for k4 in range(4):
                            wps = pp.tile(
                                [128, 512], F32, name="wps", tag=f"p{k4 % 2}"
                            )
                            col = LH * half + 512 * k4
                            nc.tensor.matmul(
                                wps,
                                lhsT,
                                wwt[32 * a : 32 * a + kk, col : col + 512],
                                start=True,
                                stop=True,
                            )
                            nc.scalar.copy(
                                out=we[:, 8 * k4 : 8 * (k4 + 1), :],
                                in_=wps[:].rearrange("p (a b) -> p a b", a=8),
                            )
                    else:
                        for j in range(4):
                            g = 4 * ct + j
                            q = 8 * d + g
                            bcast = _ap(
                                wwd[q : q + 1, :],
                                LH * half,
                                [[0, 32], [64, 32], [1, 64]],
                            )
                            bceng[j].dma_start(
                                out=we[32 * j : 32 * (j + 1), :, :], in_=bcast
                            )
                    vsrc = v2[ct][:, dy + 32 * half : dy + 32 * half + 32, dx : dx + 64]
                    if d == 0:
                        nc.vector.tensor_tensor(areg, vsrc, we[:, :, :], ALU.mult)
                        continue
                    u = upool.tile([128, 32, 64], F16, name="u", tag="uh")
                    if d in POOL_MULT:
                        nc.gpsimd.tensor_tensor(u, vsrc, we[:, :, :], ALU.mult)
                    else:
                        nc.vector.tensor_tensor(u, vsrc, we[:, :, :], ALU.mult)
                    if d < ADD_DVE:
                        nc.vector.tensor_tensor(areg, u[:, :, :], areg, ALU.add)
                    else:
                        ur = u[:].rearrange("p a b -> p (a b)")
                        for b in range(4):
                            nc.tensor.matmul(
                                hps[b],
                                ident,
                                ur[:, 512 * b : 512 * (b + 1)],
                                start=(d == ADD_DVE),
                                stop=False,
                            )
                # fold the DVE-chain accumulator into PSUM, then drain
                ar = areg.rearrange("p a b -> p (a b)")
                for b in range(4):
                    nc.tensor.matmul(
                        hps[b],
                        ident,
                        ar[:, 512 * b : 512 * (b + 1)],
                        start=False,
                        stop=True,
                    )
                for b in range(4):
                    nc.scalar.copy(
                        out=acc[ct][:, 32 * half + 8 * b : 32 * half + 8 * (b + 1), :],
                        in_=hps[b][:].rearrange("p (a b) -> p a b", a=8),
                    )
            # ---- I: final projection, split at the half boundary -------
            emit_I(range(4) if half == 0 else range(4, 8))

    if split_waits:
        _split_sync_waits(nc)
    return nc


def _prep_weights(dw_w, dw_b, om_w, om_b, vp_w, vp_b, op_w, op_b):
    perm = _om_perm()
    om_w_p = np.asarray(om_w)[perm]
    om_b_p = np.asarray(om_b)[perm]
    S = _s_matrices()

    wk9 = np.asarray(dw_w)[:, 0].reshape(C, 3, 3)
    ddg = np.zeros((128, 18, 128), np.float16)
    for tap in range(9):
        kh, kw = tap // 3, tap % 3
        for ct in range(2):
            w = wk9[128 * ct : 128 * (ct + 1), kh, kw]
            ddg[np.arange(128), 2 * tap + ct, np.arange(128)] = w.astype(np.float16)

    smat = np.zeros((72, 18, 128), np.float16)
    for ab in range(9):
        a, b = ab // 3, ab % 3
        smat[:, 2 * ab + 0, :128] = S[a, b][:, :128]
        smat[:, 2 * ab + 1, :72] = S[a, b][:, 128:]

    exsel = np.zeros((128, 8, 128), np.float16)
    for p in range(128):
        for si in range(8):
            for c in range(128):
                if (p % 32) == 4 * si + (c // 32):
                    exsel[p, si, c] = 1.0

    return {
        "exsel": exsel,
        "vp_wt": np.ascontiguousarray(np.asarray(vp_w).T).astype(np.float16),
        "vp_b2": np.ascontiguousarray(np.asarray(vp_b).reshape(2, 128).T, np.float32),
        "dw_diag": ddg,
        "dw_b2": np.ascontiguousarray(np.asarray(dw_b).reshape(2, 128).T, np.float32),
        "om_wt": np.ascontiguousarray(om_w_p.T).astype(np.float16),
        "om_b3": np.ascontiguousarray(om_b_p.reshape(3, 72).T, np.float32),
        "smat": smat,
        "op_wt": np.ascontiguousarray(np.asarray(op_w).T).astype(np.float16),
        "op_b2": np.ascontiguousarray(np.asarray(op_b).reshape(2, 128).T, np.float32),
    }


_CACHE = {}


def kernel(x, dw_w, dw_b, om_w, om_b, vp_w, vp_b, op_w, op_b, _trace=False):
    if "nc" not in _CACHE:
        _CACHE["nc"] = build_module()
    nc = _CACHE["nc"]
    wts = _prep_weights(dw_w, dw_b, om_w, om_b, vp_w, vp_b, op_w, op_b)
    x = np.asarray(x, np.float32)
    in_maps = [dict(wts, x=np.ascontiguousarray(x[n].reshape(C, L))) for n in range(N)]
    res = run_bass_kernel_spmd(nc, in_maps, list(range(NCORES)), trace=_trace)
    out = np.stack([res.results[n]["out"].reshape(C, H, W) for n in range(N)])
    if _trace:
        _CACHE["last_results"] = res
    return out
